# revision 20
# baseline (speedup 1.0000x reference)
"""Causal multi-head attention on 8 trn2 NeuronCores.

Problem: B=4, S=2048, D=1024, H=16 heads (HD=64), causal softmax attention
with out-projection + bias.

Sharding (tensor-parallel over heads, data-parallel over batch):
  core c -> batch b = c // 2, head half = c % 2 (8 of 16 heads, 512 dims).
  Every core runs the IDENTICAL program on different data:
    - xt   : x[b].T cast to bf16       [1024, 2048]
    - wq/wk/wv : W[:, half slice] bf16 [1024, 512]
    - wot  : Wo[:, half slice].T bf16  [512, 1024]
    - bot  : bias, transposed layout [128, 8] f32 (bot[p, c] = bo[c*128+p]);
             real on even cores, zeros on odd cores.
  Core output: partial TRANSPOSED out-projection [1024, 2048] in bf16; host
  upcasts, sums the two partials per batch and transposes (row-parallel
  out_proj reduction).

Kernel (per core), flash-style with transposed scores, all matmul operands
bf16 (PSUM accumulation stays fp32; rel-err gate is 2e-2):
  QT = wq.T @ x.T   [512, 2048]; KT likewise; V = x @ wv [2048, 512]
  augmented with a ones column per head (the 65th row of the ctx matmul
  then yields the softmax denominator Z).
  scoresT[k, q] per head = KT_h-slice^T @ QT_h -> psum [128 keys, q].
  Head PAIRS are computed concurrently via PE row tiling (64x128 mode,
  tile_position (0,0)/(64,0)) since the contraction dim is HD=64.
  exp on ACT (scale = 1/8 folded in) -> bf16 ex tiles; causal masking is
  applied POST-exp only on the 128-wide diagonal blocks via gpsimd
  affine_select (fill 0 where q < k), keeping both the ACT critical path
  and the DVE queue clear of mask work.
  ctx^T accumulated over key tiles (lhsT = V|ones, M=65); the softmax
  denominator is inverted with the fast DVE approx reciprocal (~5x cheaper
  than nc.vector.reciprocal), broadcast on gpsimd, multiplied on DVE into
  ct (bf16).  The transposed out-projection outT[o, q] = wot.T-chunk @ ctxT
  is interleaved into the projection trickle stream per q-chunk as soon as
  all head-blocks of that chunk are normalized, so the PE stays busy during
  the ACT-bound attention stretches and the serial tail shrinks.
"""

import os
from contextlib import ExitStack

import numpy as np
import ml_dtypes

import concourse.mybir as mybir
import concourse.tile as tile
from concourse import bacc
from concourse.bass_utils import run_bass_kernel_spmd

B, S, D, H = 4, 2048, 1024, 16
HD = 64          # head dim
DL = 512         # local head dims per core (8 heads)
HH = 8           # local heads
P = 128
QC = 512         # q chunk (moving free dim)
N_QC = S // QC   # 4
N_DI = D // P    # 8
N_DL = DL // P   # 4
N_ST = S // P    # 16 seq tiles
VW = HD + 1      # 65: V columns + ones column
O_ = 1024        # output dims (full)

F32 = mybir.dt.float32
BF16 = mybir.dt.bfloat16
N_CORES = 8


def build_nc():
    nc = bacc.Bacc("TRN2", target_bir_lowering=False, debug=False,
                   num_devices=N_CORES, num_swdge_queues=4)
    xt = nc.dram_tensor("xt", [D, S], BF16, kind="ExternalInput").ap()
    wq = nc.dram_tensor("wq", [D, DL], BF16, kind="ExternalInput").ap()
    wk = nc.dram_tensor("wk", [D, DL], BF16, kind="ExternalInput").ap()
    wv = nc.dram_tensor("wv", [D, DL], BF16, kind="ExternalInput").ap()
    wot = nc.dram_tensor("wot", [DL, O_], BF16, kind="ExternalInput").ap()
    bot = nc.dram_tensor("bot", [P, O_ // P], F32, kind="ExternalInput").ap()
    out = nc.dram_tensor("out", [O_, S], BF16, kind="ExternalOutput").ap()

    repeat = int(os.environ.get("MHA_REPEAT", "1"))
    hwloop = int(os.environ.get("MHA_HWLOOP", "0"))
    with tile.TileContext(nc) as tc:
        if hwloop > 1:
            with tc.For_i(0, hwloop, 1):
                _emit(nc, tc, xt, wq, wk, wv, wot, bot, out)
        else:
            for _ in range(repeat):
                _emit(nc, tc, xt, wq, wk, wv, wot, bot, out)
    nc.compile()
    return nc


def _emit(nc, tc, xt, wq, wk, wv, wot, bot, out):
    Exp = mybir.ActivationFunctionType.Exp
    mult = mybir.AluOpType.mult
    add = mybir.AluOpType.add

    NOMASK = bool(os.environ.get("MHA_NOMASK"))
    NOTILE = bool(os.environ.get("MHA_NOTILE"))
    STAGGER = int(os.environ.get("MHA_STAGGER", "4"))
    TRICKLE = int(os.environ.get("MHA_TRICKLE", "3"))

    with ExitStack() as ctx:
        # ---- small constants -----------------------------------------------
        consts = ctx.enter_context(tc.tile_pool(name="consts", bufs=1))
        ones_f = consts.tile([P, HH], F32, tag="ones_f")
        nc.gpsimd.memset(ones_f[:], 1.0)
        bot_sb = consts.tile([P, O_ // P], F32, tag="bot_sb")
        nc.sync.dma_start(bot_sb[:], bot[:])

        # ---- persistent storage --------------------------------------------
        qt_pool = ctx.enter_context(tc.tile_pool(name="qt", bufs=1))
        kt_pool = ctx.enter_context(tc.tile_pool(name="kt", bufs=1))
        v_pool = ctx.enter_context(tc.tile_pool(name="v", bufs=1))
        ct_pool = ctx.enter_context(tc.tile_pool(name="ct", bufs=1))
        xt_pool = ctx.enter_context(tc.tile_pool(name="xtp", bufs=1))
        w_pool = ctx.enter_context(tc.tile_pool(name="wp", bufs=1))
        wot_pool = ctx.enter_context(tc.tile_pool(name="wotp", bufs=1))
        qt_t = [qt_pool.tile([P, S], BF16, name=f"qt{j}", tag=f"qt{j}") for j in range(N_DL)]
        kt_t = [kt_pool.tile([P, S], BF16, name=f"kt{j}", tag=f"kt{j}") for j in range(N_DL)]
        v_t = [v_pool.tile([P, HH * VW], BF16, name=f"v{i}", tag=f"v{i}") for i in range(N_ST)]
        ct_t = [ct_pool.tile([P, S], BF16, name=f"ct{j}", tag=f"ct{j}") for j in range(N_DL)]

        # input DMAs in consumption order: the first attention unit needs all
        # of wq/wk/wv/xt, so interleave per-di on two HWDGE queues (weights on
        # the scalar queue, xt on sync); wot arrives last (out-proj).
        # Batched input loads: ONE DMA trigger per weight matrix (vs one per
        # 128-row chunk).  HWDGE triggers cost ~0.6us of issuing-engine time
        # each, and at the hw-loop boundary they serialize after the engine's
        # last compute op — 28 triggers were adding ~11us of PE stall per
        # iteration.  The folded [P, di*cols] SBUF layout keeps every
        # downstream slice identical via AP views.
        # wq/wk ride the sync queue AHEAD of xt: the sync engine goes idle
        # ~10us before the hw-loop boundary, while the scalar engine is still
        # issuing the tail's qc3 output writes — putting the first-needed
        # weights on sync pulls the next iteration's first matmul ~8us
        # earlier.  wv/wot (needed later) stay on the scalar queue.
        w_big = {}
        for nm, w_dram in (("q", wq), ("k", wk), ("v", wv)):
            wb = w_pool.tile([P, N_DI * DL], BF16, name=f"w{nm}", tag=f"w{nm}")
            eng = nc.scalar if nm == "v" else nc.sync
            eng.dma_start(
                wb.rearrange("p (di c) -> p di c", c=DL),
                w_dram.rearrange("(di p) c -> p di c", p=P))
            w_big[nm] = wb
        xt_big = xt_pool.tile([P, N_DI * S], BF16, name="xtb", tag="xtb")
        # column halves: the first qk/v chunks only touch q < 1024, so the
        # second half can trail without gating the pipeline start.
        for h in (0, 1):
            nc.sync.dma_start(
                xt_big.rearrange("p (di c) -> p di c", c=S)
                      [:, :, h * (S // 2):(h + 1) * (S // 2)],
                xt.rearrange("(di p) c -> p di c", p=P)
                  [:, :, h * (S // 2):(h + 1) * (S // 2)])
        wot_big = wot_pool.tile([P, N_DL * O_], BF16, name="wotb", tag="wotb")
        nc.scalar.dma_start(
            wot_big.rearrange("p (j o) -> p j o", o=O_),
            wot.rearrange("(j p) o -> p j o", p=P))
        xt_sb = [xt_big[:, i * S:(i + 1) * S] for i in range(N_DI)]
        w_sb = {(nm, i): w_big[nm][:, i * DL:(i + 1) * DL]
                for nm in ("q", "k", "v") for i in range(N_DI)}
        wot_sb = [wot_big[:, j * O_:(j + 1) * O_] for j in range(N_DL)]

        pps = ctx.enter_context(tc.tile_pool(name="pps", bufs=2, space="PSUM"))
        exp_pool = ctx.enter_context(tc.tile_pool(
            name="exp", bufs=int(os.environ.get("MHA_EXBUFS", "12"))))
        z_pool = ctx.enter_context(tc.tile_pool(name="zp", bufs=4))
        sc_pool = ctx.enter_context(tc.tile_pool(name="scps", bufs=2, space="PSUM"))
        ctx_pool = ctx.enter_context(tc.tile_pool(name="ctxps", bufs=1, space="PSUM"))
        out_pool = ctx.enter_context(tc.tile_pool(name="outp", bufs=4))

        # ---- projection / out-projection substep stream --------------------
        # Substeps of ~2 matmuls each; attention emission drains/trickles them
        # so the PE never idles during the ACT-bound attention stretches.
        proj_steps = []
        marks = {}
        live = {}

        def make_qk_steps(nm, dst, dq, qch):
            for di in range(N_DI):
                def step(nm=nm, dst=dst, dq=dq, qch=qch, di=di):
                    key = (nm, dq, qch)
                    if di == 0:
                        live[key] = [
                            pps.tile([P, QC], F32, tag="pp",
                                     name=f"pp_{nm}{dq}_{qch}_{i}")
                            for i in (0, 1)]
                    ps2 = live[key]
                    for i, qc in enumerate((2 * qch, 2 * qch + 1)):
                        nc.tensor.matmul(
                            ps2[i][:],
                            w_sb[nm, di][:, dq * P:(dq + 1) * P],
                            xt_sb[di][:, qc * QC:(qc + 1) * QC],
                            start=(di == 0), stop=(di == N_DI - 1))
                proj_steps.append(step)

            def copy_step(nm=nm, dst=dst, dq=dq, qch=qch):
                ps2 = live.pop((nm, dq, qch))
                for i, qc in enumerate((2 * qch, 2 * qch + 1)):
                    nc.vector.tensor_copy(
                        dst[dq][:, qc * QC:(qc + 1) * QC], ps2[i][:])
            proj_steps.append(copy_step)

        def make_v_steps(sg):
            for sth in (0, 1):
                sts = (4 * sg + 2 * sth, 4 * sg + 2 * sth + 1)
                for di in range(N_DI):
                    def step(sth=sth, di=di, sts=sts, sg=sg):
                        key = ("v", sg, sth)
                        if di == 0:
                            live[key] = [
                                pps.tile([P, DL], F32, tag="pp",
                                         name=f"ppv{sg}_{sth}_{i}")
                                for i in (0, 1)]
                        ps2 = live[key]
                        for i, st in enumerate(sts):
                            nc.tensor.matmul(
                                ps2[i][:],
                                xt_sb[di][:, st * P:(st + 1) * P],
                                w_sb["v", di][:],
                                start=(di == 0), stop=(di == N_DI - 1))
                    proj_steps.append(step)

                def copy_step(sth=sth, sts=sts, sg=sg):
                    ps2 = live.pop(("v", sg, sth))
                    for i, st in enumerate(sts):
                        vv = v_t[st].rearrange("p (h w) -> p h w", w=VW)
                        nc.vector.tensor_copy(
                            vv[:, :, 0:HD],
                            ps2[i].rearrange("p (h w) -> p h w", w=HD))
                        nc.vector.tensor_copy(
                            vv[:, :, HD:VW],
                            ones_f.rearrange("p (h o) -> p h o", o=1))
                proj_steps.append(copy_step)

        # Emit projection chunks in exactly the order the attention staircase
        # first needs them, so drains stay minimal and the hw-loop boundary
        # only waits on the first qk chunk's inputs.
        for item in (("qk", 0, 0), ("v", 0), ("qk", 1, 0), ("v", 1),
                     ("qk", 2, 0), ("qk", 0, 1), ("v", 2), ("qk", 3, 0),
                     ("qk", 1, 1), ("v", 3), ("qk", 2, 1), ("qk", 3, 1)):
            if item[0] == "qk":
                _, dq, qch = item
                make_qk_steps("q", qt_t, dq, qch)
                make_qk_steps("k", kt_t, dq, qch)
                marks["qk", dq, qch] = len(proj_steps)
            else:
                make_v_steps(item[1])
                marks["v", item[1]] = len(proj_steps)

        def make_outproj_steps(qc):
            # transposed out-projection for one q-chunk; appended to the step
            # stream once all N_DL head-blocks of chunk qc are normalized.
            for oc in range(O_ // P):
                for half in (0, 1):
                    def mm_step(oc=oc, half=half, qc=qc):
                        key = ("op", qc, oc)
                        if half == 0:
                            live[key] = pps.tile(
                                [P, QC], F32, tag="pp", name=f"op{qc}_{oc}")
                        ps = live[key]
                        for dl in (2 * half, 2 * half + 1):
                            nc.tensor.matmul(
                                ps[:],
                                wot_sb[dl][:, oc * P:(oc + 1) * P],
                                ct_t[dl][:, qc * QC:(qc + 1) * QC],
                                start=(dl == 0), stop=(dl == N_DL - 1))
                    proj_steps.append(mm_step)

                def evict_step(oc=oc, qc=qc):
                    ps = live.pop(("op", qc, oc))
                    ob = out_pool.tile([P, QC], BF16, tag="ob",
                                       name=f"ob{qc}_{oc}")
                    nc.vector.tensor_scalar(
                        ob[:], ps[:], bot_sb[:, oc:oc + 1], None, add)
                    # HWDGE queues only: SWDGE (gpsimd) outs were measured
                    # slower (descriptor-path latency stalls the ob pool).
                    # The final q-chunk's writes go on the scalar queue (ACT
                    # is idle by then) so the next hw-loop iteration's xt
                    # prefetch on sync isn't stuck behind them.
                    eng = nc.scalar if qc == N_QC - 1 else nc.sync
                    eng.dma_start(
                        out[oc * P:(oc + 1) * P, qc * QC:(qc + 1) * QC],
                        ob[:])
                proj_steps.append(evict_step)

        pi = [0]

        def drain_to(idx):
            while pi[0] < idx:
                proj_steps[pi[0]]()
                pi[0] += 1

        def trickle(n):
            for _ in range(n):
                if pi[0] < len(proj_steps):
                    proj_steps[pi[0]]()
                    pi[0] += 1

        # ---- attention unit stream -----------------------------------------
        # Staircase order over (pr, qc) so projection chunks are consumed
        # evenly; groups g = key-tile pairs within a unit.
        unit_order = sorted(
            ((pr, qc) for pr in range(N_DL) for qc in range(N_QC)),
            key=lambda u: (u[0] + u[1], u[1]))
        units = []
        for pr, qc in unit_order:
            ng = 2 * (qc + 1)
            for g in range(ng):
                units.append((pr, qc, g, ng))

        state = {}
        qc_done = [0] * N_QC

        def emit_scores(u):
            pr, qc, g, ng = u
            if g == 0:
                drain_to(marks["qk", pr, qc // 2])
                drain_to(marks["v", qc])
            sc2 = [sc_pool.tile([P, 2 * QC], F32, tag="sc",
                                name=f"sc{hi}_{pr}_{qc}_{g}")
                   for hi in (0, 1)]
            offs = []
            for j in (0, 1):
                kt = 2 * g + j
                d = max(0, kt * P - qc * QC)   # masked q prefix width
                offs.append(d)
                for hi in (0, 1):
                    bp = 64 * hi
                    nc.tensor.matmul(
                        sc2[hi][:, j * QC + d:(j + 1) * QC],
                        kt_t[pr][bp:bp + HD, kt * P:(kt + 1) * P],
                        qt_t[pr][bp:bp + HD, qc * QC + d:(qc + 1) * QC],
                        start=True, stop=True,
                        tile_position=None if NOTILE else (bp, 0))
            ex2 = [exp_pool.tile([P, 2 * QC], BF16, tag="ex",
                                 name=f"ex{hi}_{pr}_{qc}_{g}")
                   for hi in (0, 1)]
            for hi in (0, 1):
                if offs[0] == offs[1]:
                    nc.scalar.activation(ex2[hi][:, offs[0]:2 * QC],
                                         sc2[hi][:, offs[0]:2 * QC],
                                         Exp, scale=0.125)
                else:
                    for j in (0, 1):
                        d = offs[j]
                        nc.scalar.activation(
                            ex2[hi][:, j * QC + d:(j + 1) * QC],
                            sc2[hi][:, j * QC + d:(j + 1) * QC],
                            Exp, scale=0.125)
            if not NOMASK:
                # post-exp causal zeroing of the 128-wide diagonal block:
                # keep ex[k, q'] (q' local to the block) where q' - k >= 0.
                for j in (0, 1):
                    kt = 2 * g + j
                    dd = kt * P - qc * QC
                    if 0 <= dd < QC:  # kt on the diagonal band of qc
                        col = j * QC + dd
                        for hi in (0, 1):
                            nc.gpsimd.affine_select(
                                out=ex2[hi][:, col:col + P],
                                in_=ex2[hi][:, col:col + P],
                                pattern=[[1, P]],
                                compare_op=mybir.AluOpType.is_ge,
                                fill=0.0, base=0, channel_multiplier=-1)
            state[(pr, qc, g)] = (ex2, offs)

        def emit_ctx(u):
            pr, qc, g, ng = u
            if g == 0:
                state[(pr, qc, "ctx")] = [
                    ctx_pool.tile([VW, QC], F32, tag=f"ctx{hi}",
                                  name=f"ctx{hi}_{pr}_{qc}")
                    for hi in (0, 1)]
            ctx2 = state[(pr, qc, "ctx")]
            ex2, offs = state.pop((pr, qc, g))
            nkt = 2 * ng
            for j in (0, 1):
                kt = 2 * g + j
                d = offs[j]
                for hi in (0, 1):
                    h = 2 * pr + hi
                    nc.tensor.matmul(
                        ctx2[hi][0:VW, d:QC],
                        v_t[kt][:, h * VW:(h + 1) * VW],
                        ex2[hi][:, j * QC + d:(j + 1) * QC],
                        start=(kt == 0), stop=(kt == nkt - 1))
            if g == ng - 1:
                ctx2 = state.pop((pr, qc, "ctx"))
                if (os.environ.get("MHA_RECIP", "batch") == "exact"
                        or (pr, qc) == (N_DL - 1, N_QC - 1)):
                    # v1-validated direct path: per-hi [1, 512] reciprocal
                    # straight from PSUM (slow: ~3.3us of DVE each).  Also
                    # used for the final (pr, qc) group: its normalize chain
                    # gates the tail out-projection, and the direct path has
                    # ~3us less latency than the DMA-batched one.
                    for hi in (0, 1):
                        bp = 64 * hi
                        rec = z_pool.tile([1, QC], F32, tag="rec")
                        nc.vector.reciprocal(rec[:], ctx2[hi][HD:VW, :])
                        rzb = z_pool.tile([HD, QC], F32, tag="rzb")
                        nc.gpsimd.partition_broadcast(rzb[:], rec[:])
                        nc.vector.tensor_tensor(
                            ct_t[pr][bp:bp + HD, qc * QC:(qc + 1) * QC],
                            ctx2[hi][0:HD, :], rzb[:], mult)
                else:
                    # Batched denominator path: evict ctx+Z to SBUF (frees
                    # PSUM fast), DMA the two [1,512] Z rows into a [8,128]
                    # layout so ONE exact reciprocal covers both heads at
                    # 128 elems/lane, DMA back, broadcast, multiply.
                    cu2 = []
                    for hi in (0, 1):
                        cu = z_pool.tile([VW, QC], F32, tag=f"cu{hi}",
                                         bufs=2, name=f"cu{hi}_{pr}_{qc}")
                        nc.vector.tensor_copy(cu[:], ctx2[hi][:])
                        cu2.append(cu)
                    zb = z_pool.tile([8, P], F32, tag="zb")
                    for hi in (0, 1):
                        nc.sync.dma_start(
                            zb[4 * hi:4 * hi + 4, :], cu2[hi][HD:VW, :])
                    zr = z_pool.tile([8, P], F32, tag="zr")
                    nc.vector.reciprocal(zr[:], zb[:])
                    for hi in (0, 1):
                        bp = 64 * hi
                        rzs = z_pool.tile([1, QC], F32, tag="rzs")
                        nc.sync.dma_start(rzs[:], zr[4 * hi:4 * hi + 4, :])
                        rzb = z_pool.tile([HD, QC], F32, tag="rzb")
                        nc.gpsimd.partition_broadcast(rzb[:], rzs[:])
                        nc.vector.tensor_tensor(
                            ct_t[pr][bp:bp + HD, qc * QC:(qc + 1) * QC],
                            cu2[hi][0:HD, :], rzb[:], mult)
                qc_done[qc] += 1
                if qc_done[qc] == N_DL:
                    make_outproj_steps(qc)

        for i, u in enumerate(units):
            emit_scores(u)
            trickle(TRICKLE)
            if i >= STAGGER:
                emit_ctx(units[i - STAGGER])
        for u in units[-STAGGER:]:
            emit_ctx(u)
            trickle(TRICKLE)
        drain_to(len(proj_steps))


_NC_CACHE = None


def _get_nc():
    global _NC_CACHE
    if _NC_CACHE is None:
        _NC_CACHE = build_nc()
    return _NC_CACHE


def make_in_maps(x, Wq, Wk, Wv, Wo, bo):
    bf = ml_dtypes.bfloat16
    in_maps = []
    xts = [np.ascontiguousarray(x[b].T).astype(bf) for b in range(B)]
    bot = np.ascontiguousarray(bo.reshape(O_ // P, P).T).astype(np.float32)
    zeros_bot = np.zeros((P, O_ // P), np.float32)
    for c in range(N_CORES):
        b, half = c // 2, c % 2
        d0 = half * DL
        in_maps.append({
            "xt": xts[b],
            "wq": np.ascontiguousarray(Wq[:, d0:d0 + DL]).astype(bf),
            "wk": np.ascontiguousarray(Wk[:, d0:d0 + DL]).astype(bf),
            "wv": np.ascontiguousarray(Wv[:, d0:d0 + DL]).astype(bf),
            "wot": np.ascontiguousarray(Wo[:, d0:d0 + DL].T).astype(bf),
            "bot": bot if half == 0 else zeros_bot,
        })
    return in_maps


def kernel(x, Wq, Wk, Wv, Wo, bo):
    x = np.asarray(x, np.float32)
    Wq = np.asarray(Wq, np.float32)
    Wk = np.asarray(Wk, np.float32)
    Wv = np.asarray(Wv, np.float32)
    Wo = np.asarray(Wo, np.float32)
    bo = np.asarray(bo, np.float32)
    nc = _get_nc()
    in_maps = make_in_maps(x, Wq, Wk, Wv, Wo, bo)
    res = run_bass_kernel_spmd(nc, in_maps, core_ids=list(range(N_CORES)))
    out = np.empty((B, S, O_), np.float32)
    for b in range(B):
        out[b] = (res.results[2 * b]["out"].astype(np.float32)
                  + res.results[2 * b + 1]["out"].astype(np.float32)).T
    return out
